# revision 7
# baseline (speedup 1.0000x reference)
"""Trainium2 Bass kernel for an ActorCritic model (CNN + enc GRU + autoregressive
dec GRU + linear heads), data-parallel across 8 NeuronCores.

Layout choices (per core, B_local=1024):
  - hidden/gate dim (128) on SBUF partitions, batch on the free dim
  - batch processed in 2 chunks of 512 so gate psum tiles fill whole banks
    and the two chunks pipeline across TensorE / ScalarE / VectorE
  - fp16 compute for matmuls + VectorE tensor_tensor ops (2x DVE mode),
    fp32 psum accumulation, fp32 output
  - all biases folded into either matmul contract rows (encoder x-side via a
    ones row), ScalarE activation bias (per-partition), or the fused
    scalar_tensor_tensor op (hn + bhh_n)*r
"""

import numpy as np

H = 128
T = 64
NCORES = 8
B = 8192
BL = B // NCORES          # 1024 batch per core
BC = 512                  # batch chunk (psum bank = 512 fp32)
NCH = (BL // BC)          # chunks
N9 = BL * 9               # conv1 moving columns (pos-major)

_CACHE = {}


def _build_program():
    import concourse.bacc as bacc
    import concourse.tile as tile
    import concourse.mybir as mybir

    f16 = mybir.dt.float16
    f32 = mybir.dt.float32
    AF = mybir.ActivationFunctionType
    ALU = mybir.AluOpType

    nc = bacc.Bacc("TRN2", target_bir_lowering=False, debug=False)

    # ---------------- DRAM tensors ----------------
    xs_d = nc.dram_tensor("xs", [T * 30, BL], f16, kind="ExternalInput").ap()
    cnn0_d = nc.dram_tensor("cnn0", [128, N9], f16, kind="ExternalInput").ap()
    cnn1_d = nc.dram_tensor("cnn1", [115, N9], f16, kind="ExternalInput").ap()
    w1a_d = nc.dram_tensor("w1a", [128, 16], f16, kind="ExternalInput").ap()
    w1b_d = nc.dram_tensor("w1b", [115, 16], f16, kind="ExternalInput").ap()
    w2_d = nc.dram_tensor("w2", [16, 128], f16, kind="ExternalInput").ap()
    fcw_d = nc.dram_tensor("fcw", [128, 128], f16, kind="ExternalInput").ap()
    ewih_d = nc.dram_tensor("ewih", [30, 384], f16, kind="ExternalInput").ap()
    ewhh_d = nc.dram_tensor("ewhh", [128, 384], f16, kind="ExternalInput").ap()
    dw1_d = nc.dram_tensor("dw1", [128, 256], f16, kind="ExternalInput").ap()
    dwnx_d = nc.dram_tensor("dwnx", [128, 128], f16, kind="ExternalInput").ap()
    dwnh_d = nc.dram_tensor("dwnh", [128, 128], f16, kind="ExternalInput").ap()
    dwhhrz_d = nc.dram_tensor("dwhhrz", [128, 256], f16, kind="ExternalInput").ap()
    hcnn_d = nc.dram_tensor("hcnn", [128, 79], f16, kind="ExternalInput").ap()
    hpn_d = nc.dram_tensor("hpn", [128, 79], f16, kind="ExternalInput").ap()
    # biases (f32, column vectors)
    c1b_d = nc.dram_tensor("c1b", [16, 1], f32, kind="ExternalInput").ap()
    c2b_d = nc.dram_tensor("c2b", [32, 1], f32, kind="ExternalInput").ap()
    fcb_d = nc.dram_tensor("fcb", [128, 1], f32, kind="ExternalInput").ap()
    ebhn_d = nc.dram_tensor("ebhn", [128, 1], f32, kind="ExternalInput").ap()
    dbr_d = nc.dram_tensor("dbr", [128, 1], f32, kind="ExternalInput").ap()
    dbz_d = nc.dram_tensor("dbz", [128, 1], f32, kind="ExternalInput").ap()
    dbxn_d = nc.dram_tensor("dbxn", [128, 1], f32, kind="ExternalInput").ap()
    dbhn_d = nc.dram_tensor("dbhn", [128, 1], f32, kind="ExternalInput").ap()
    hb_d = nc.dram_tensor("hb", [79, 1], f32, kind="ExternalInput").ap()

    out_d = nc.dram_tensor("out", [79, BL], f32, kind="ExternalOutput").ap()

    with tile.TileContext(nc) as tc:
        with tc.tile_pool(name="const", bufs=1) as cpool, \
             tc.tile_pool(name="persist", bufs=1) as ppool, \
             tc.tile_pool(name="hbuf", bufs=2) as hpool, \
             tc.tile_pool(name="xbuf", bufs=4) as xpool, \
             tc.tile_pool(name="gates", bufs=3) as gpool:

            def load_const(ap_d, shape, dt, tag):
                t = cpool.tile(shape, dt, tag=tag)
                nc.sync.dma_start(t[:, :], ap_d)
                return t

            w1a = load_const(w1a_d, [128, 16], f16, "w1a")
            w1b = load_const(w1b_d, [115, 16], f16, "w1b")
            w2 = load_const(w2_d, [16, 128], f16, "w2")
            fcw = load_const(fcw_d, [128, 128], f16, "fcw")
            ewih = load_const(ewih_d, [30, 384], f16, "ewih")
            ewhh = load_const(ewhh_d, [128, 384], f16, "ewhh")
            dw1 = load_const(dw1_d, [128, 256], f16, "dw1")
            dwnx = load_const(dwnx_d, [128, 128], f16, "dwnx")
            dwnh = load_const(dwnh_d, [128, 128], f16, "dwnh")
            dwhhrz = load_const(dwhhrz_d, [128, 256], f16, "dwhhrz")
            hcnn = load_const(hcnn_d, [128, 79], f16, "hcnn")
            hpn = load_const(hpn_d, [128, 79], f16, "hpn")
            c1b = load_const(c1b_d, [16, 1], f32, "c1b")
            c2b = load_const(c2b_d, [32, 1], f32, "c2b")
            fcb = load_const(fcb_d, [128, 1], f32, "fcb")
            ebhn = load_const(ebhn_d, [128, 1], f32, "ebhn")
            dbr = load_const(dbr_d, [128, 1], f32, "dbr")
            dbz = load_const(dbz_d, [128, 1], f32, "dbz")
            dbxn = load_const(dbxn_d, [128, 1], f32, "dbxn")
            dbhn = load_const(dbhn_d, [128, 1], f32, "dbhn")
            hbias = load_const(hb_d, [79, 1], f32, "hb")

            y1 = ppool.tile([16, N9], f16, tag="y1")        # conv1 relu out
            m2 = ppool.tile([128, BL], f16, tag="m2")       # conv2 out (fc in)
            zcnn = ppool.tile([128, BL], f16, tag="zcnn")   # fc relu out

            # ================= CNN =================
            with tc.tile_pool(name="cnnin", bufs=3) as cin, \
                 tc.tile_pool(name="cnnps", bufs=2, space="PSUM") as cps:
                # conv1: im2col matmul, contract 243 = 128 + 115
                CC = 512
                relu_flip = 0
                for j in range(0, N9, CC):
                    w = min(CC, N9 - j)
                    a0 = cin.tile([128, CC], f16, tag="a0")
                    a1 = cin.tile([115, CC], f16, tag="a1")
                    nc.sync.dma_start(a0[:, :w], cnn0_d[:, j:j + w])
                    nc.sync.dma_start(a1[:, :w], cnn1_d[:, j:j + w])
                    p1 = cps.tile([16, CC], f32, tag="p1")
                    nc.tensor.matmul(p1[:, :w], w1a[:, :], a0[:, :w],
                                     start=True, stop=False)
                    nc.tensor.matmul(p1[:, :w], w1b[:, :], a1[:, :w],
                                     start=False, stop=True)
                    if relu_flip:
                        nc.scalar.activation(y1[:, j:j + w], p1[:, :w],
                                             AF.Relu, bias=c1b[:, :])
                    else:
                        nc.vector.tensor_scalar(y1[:, j:j + w], p1[:, :w],
                                                scalar1=c1b[:, :], scalar2=0.0,
                                                op0=ALU.add, op1=ALU.max)
                    relu_flip ^= 1

                # conv2: 2x2 taps accumulated; moving = contiguous y1 slices
                for q in range(4):          # output pixel (qy, qx)
                    qy, qx = q // 2, q % 2
                    for cb in range(NCH):
                        b0 = cb * BC
                        p2 = cps.tile([32, BC], f32, tag="p2")
                        for k in range(4):  # tap (ky, kx)
                            ky, kx = k // 2, k % 2
                            pix = (qy + ky) * 3 + (qx + kx)
                            nc.tensor.matmul(
                                p2[:, :], w2[:, 32 * k:32 * k + 32],
                                y1[:, pix * BL + b0: pix * BL + b0 + BC],
                                start=(k == 0), stop=(k == 3))
                        dst = m2[32 * q:32 * q + 32, b0:b0 + BC]
                        if q % 2:
                            nc.scalar.activation(dst, p2[:, :], AF.Relu,
                                                 bias=c2b[:, :])
                        else:
                            nc.vector.tensor_scalar(dst, p2[:, :],
                                                    scalar1=c2b[:, :],
                                                    scalar2=0.0,
                                                    op0=ALU.add, op1=ALU.max)

                # fc 128->128 + relu
                for cb in range(NCH):
                    b0 = cb * BC
                    pf = cps.tile([128, BC], f32, tag="pf")
                    nc.tensor.matmul(pf[:, :], fcw[:, :], m2[:, b0:b0 + BC],
                                     start=True, stop=True)
                    nc.scalar.activation(zcnn[:, b0:b0 + BC], pf[:, :],
                                         AF.Relu, bias=fcb[:, :])

            # ================= GRU =================
            h = hpool.tile([128, BL], f16, tag="h")
            nc.vector.memset(h[:, :], 0.0)

            with tc.tile_pool(name="grups", bufs=2, space="PSUM") as gps:

                def gru_step(h_prev, h_next, xt, phase, t):
                    """one GRU step over both batch chunks.

                    phase: 'enc' | 'dec0' | 'dec'
                    """
                    for cb in range(NCH):
                        b0 = cb * BC
                        hs = h_prev[:, b0:b0 + BC]
                        rz = gps.tile([128, 2 * BC], f32, tag="rz")
                        nx = gps.tile([128, 2 * BC], f32, tag="nx")
                        if phase == 'enc':
                            xc = xt[:, b0:b0 + BC]
                            nc.tensor.matmul(rz[:, :BC], ewih[:, 0:128], xc,
                                             start=True, stop=False)
                            nc.tensor.matmul(rz[:, :BC], ewhh[:, 0:128], hs,
                                             start=False, stop=True)
                            nc.tensor.matmul(rz[:, BC:], ewih[:, 128:256], xc,
                                             start=True, stop=False)
                            nc.tensor.matmul(rz[:, BC:], ewhh[:, 128:256], hs,
                                             start=False, stop=True)
                            nc.tensor.matmul(nx[:, :BC], ewih[:, 256:384], xc,
                                             start=True, stop=True)
                            nc.tensor.matmul(nx[:, BC:], ewhh[:, 256:384], hs,
                                             start=True, stop=True)
                        elif phase == 'dec0':
                            nc.tensor.matmul(rz[:, :BC], dwhhrz[:, 0:128], hs,
                                             start=True, stop=True)
                            nc.tensor.matmul(rz[:, BC:], dwhhrz[:, 128:256], hs,
                                             start=True, stop=True)
                            nc.tensor.matmul(nx[:, BC:], dwnh[:, :], hs,
                                             start=True, stop=True)
                        else:
                            nc.tensor.matmul(rz[:, :BC], dw1[:, 0:128], hs,
                                             start=True, stop=True)
                            nc.tensor.matmul(rz[:, BC:], dw1[:, 128:256], hs,
                                             start=True, stop=True)
                            nc.tensor.matmul(nx[:, :BC], dwnx[:, :], hs,
                                             start=True, stop=True)
                            nc.tensor.matmul(nx[:, BC:], dwnh[:, :], hs,
                                             start=True, stop=True)

                        # gates
                        rzs = gpool.tile([128, 2 * BC], f16, tag="rzs")
                        if phase == 'enc':
                            nc.scalar.activation(rzs[:, :], rz[:, :], AF.Sigmoid)
                        else:
                            nc.scalar.activation(rzs[:, :BC], rz[:, :BC],
                                                 AF.Sigmoid, bias=dbr[:, :])
                            nc.scalar.activation(rzs[:, BC:], rz[:, BC:],
                                                 AF.Sigmoid, bias=dbz[:, :])
                        bhn = ebhn if phase == 'enc' else dbhn
                        tt = gpool.tile([128, BC], f16, tag="tt")
                        nc.vector.scalar_tensor_tensor(
                            tt[:, :], nx[:, BC:], bhn[:, :], rzs[:, :BC],
                            op0=ALU.add, op1=ALU.mult)
                        nn = gpool.tile([128, BC], f16, tag="nn")
                        if phase == 'dec0':
                            nc.scalar.activation(nn[:, :], tt[:, :], AF.Tanh,
                                                 bias=dbxn[:, :])
                        else:
                            uu = gpool.tile([128, BC], f16, tag="uu")
                            nc.vector.tensor_add(uu[:, :], tt[:, :], nx[:, :BC])
                            if phase == 'enc':
                                nc.scalar.activation(nn[:, :], uu[:, :], AF.Tanh)
                            else:
                                nc.scalar.activation(nn[:, :], uu[:, :], AF.Tanh,
                                                     bias=dbxn[:, :])
                        vv = gpool.tile([128, BC], f16, tag="vv")
                        nc.vector.tensor_sub(vv[:, :], hs, nn[:, :])
                        ww = gpool.tile([128, BC], f16, tag="ww")
                        nc.vector.tensor_mul(ww[:, :], vv[:, :], rzs[:, BC:])
                        nc.vector.tensor_add(h_next[:, b0:b0 + BC], nn[:, :],
                                             ww[:, :])

                # encoder
                for t in range(T):
                    xt = xpool.tile([30, BL], f16, tag="x")
                    nc.sync.dma_start(xt[:, :], xs_d[30 * t:30 * t + 30, :])
                    h_next = hpool.tile([128, BL], f16, tag="h")
                    gru_step(h, h_next, xt, 'enc', t)
                    h = h_next

                # decoder (autoregressive; input == previous h)
                for t in range(T):
                    h_next = hpool.tile([128, BL], f16, tag="h")
                    gru_step(h, h_next, None, 'dec0' if t == 0 else 'dec', t)
                    h = h_next

            # ================= heads =================
            with tc.tile_pool(name="headps", bufs=2, space="PSUM") as hps, \
                 tc.tile_pool(name="headout", bufs=2) as opool:
                for cb in range(NCH):
                    b0 = cb * BC
                    ph = hps.tile([79, BC], f32, tag="ph")
                    nc.tensor.matmul(ph[:, :], hcnn[:, :], zcnn[:, b0:b0 + BC],
                                     start=True, stop=False)
                    nc.tensor.matmul(ph[:, :], hpn[:, :], h[:, b0:b0 + BC],
                                     start=False, stop=True)
                    os_ = opool.tile([79, BC], f32, tag="os")
                    nc.scalar.activation(os_[:, :], ph[:, :], AF.Identity,
                                         bias=hbias[:, :])
                    nc.sync.dma_start(out_d[:, b0:b0 + BC], os_[:, :])

    nc.compile()
    return nc


def _prep_in_maps(inputs):
    f16 = np.float16
    f32 = np.float32
    g = {k: np.asarray(v, f32) for k, v in inputs.items()}

    # ---- shared weight transforms ----
    w1 = g['conv1_w'].transpose(2, 3, 1, 0).reshape(243, 16)  # (ky,kx,c),o
    # taps side-by-side in columns: w2[c, k*32+o] = conv2_w[o,c,ky,kx], k=ky*2+kx
    w2 = g['conv2_w'].transpose(2, 3, 1, 0).reshape(4, 16, 32)
    w2 = w2.transpose(1, 0, 2).reshape(16, 128)
    perm = np.empty(128, np.int64)
    for q in range(4):
        for o in range(32):
            perm[q * 32 + o] = o * 4 + q
    fcw = g['fc_w'][:, perm].T                                # (128,128)

    brow = g['enc_bih'].copy()
    brow[:256] += g['enc_bhh'][:256]
    ewih = np.concatenate([g['enc_Wih'].T, brow[None, :]], axis=0)  # (30,384)
    ewhh = g['enc_Whh'].T                                           # (128,384)

    dw1 = (g['dec_Wih'][:256] + g['dec_Whh'][:256]).T          # (128,256)
    dwnx = g['dec_Wih'][256:384].T
    dwnh = g['dec_Whh'][256:384].T
    dwhhrz = g['dec_Whh'][:256].T

    whead = np.concatenate([g['dis_w'], g['value_w']], axis=0)  # (79,256)
    hcnn = whead[:, :128].T
    hpn = whead[:, 128:].T
    hb = np.concatenate([g['dis_b'], g['value_b']])[:, None]

    shared = {
        'w1a': w1[:128].astype(f16), 'w1b': w1[128:].astype(f16),
        'w2': w2.astype(f16), 'fcw': fcw.astype(f16),
        'ewih': ewih.astype(f16), 'ewhh': ewhh.astype(f16),
        'dw1': dw1.astype(f16), 'dwnx': dwnx.astype(f16),
        'dwnh': dwnh.astype(f16), 'dwhhrz': dwhhrz.astype(f16),
        'hcnn': hcnn.astype(f16), 'hpn': hpn.astype(f16),
        'c1b': g['conv1_b'][:, None].astype(f32),
        'c2b': g['conv2_b'][:, None].astype(f32),
        'fcb': g['fc_b'][:, None].astype(f32),
        'ebhn': g['enc_bhh'][256:384][:, None].astype(f32),
        'dbr': (g['dec_bih'][:128] + g['dec_bhh'][:128])[:, None].astype(f32),
        'dbz': (g['dec_bih'][128:256] + g['dec_bhh'][128:256])[:, None].astype(f32),
        'dbxn': g['dec_bih'][256:384][:, None].astype(f32),
        'dbhn': g['dec_bhh'][256:384][:, None].astype(f32),
        'hb': hb.astype(f32),
    }

    cnn = g['cnn_states']          # (B, 8, 8, 27)
    lin = g['linears_states']      # (B, T, 29)
    in_maps = []
    for c in range(NCORES):
        s = slice(c * BL, (c + 1) * BL)
        cnn_c = cnn[s]
        lin_c = lin[s]
        # im2col, pos-major columns: col = pos*BL + b, row = (ky*3+kx)*27 + c
        col = np.empty((243, 9, BL), f32)
        for oy in range(3):
            for ox in range(3):
                patch = cnn_c[:, 2 * oy:2 * oy + 3, 2 * ox:2 * ox + 3, :]
                col[:, oy * 3 + ox, :] = patch.reshape(BL, 243).T
        col = col.reshape(243, N9)
        xs = np.empty((T, 30, BL), f32)
        xs[:, :29, :] = lin_c.transpose(1, 2, 0)
        xs[:, 29, :] = 1.0
        m = dict(shared)
        m['cnn0'] = col[:128].astype(f16)
        m['cnn1'] = col[128:].astype(f16)
        m['xs'] = xs.reshape(T * 30, BL).astype(f16)
        in_maps.append(m)
    return in_maps


def _get_program():
    if 'nc' not in _CACHE:
        _CACHE['nc'] = _build_program()
    return _CACHE['nc']


def kernel(**inputs):
    from concourse.bass_utils import run_bass_kernel_spmd

    nc = _get_program()
    in_maps = _prep_in_maps(inputs)
    res = run_bass_kernel_spmd(nc, in_maps, core_ids=list(range(NCORES)))
    outs = [res.results[c]['out'] for c in range(NCORES)]    # (79, BL) each
    full = np.concatenate(outs, axis=1).T.astype(np.float32)  # (B, 79)
    distris = np.ascontiguousarray(full[:, :78])
    value = np.ascontiguousarray(full[:, 78:79])
    return distris, value
